# revision 18
# baseline (speedup 1.0000x reference)
"""Trainium2 Bass kernel for nn_Grid_fun: out = tile(feat(z), 6) @ a.

Math: z = [x, 1] (N,4); feat = (z otimes z).reshape(N,16); out = tile(feat,6) @ a
    = feat @ a_eff  where a_eff = a.reshape(6,16,3).sum(0)   [16,3]
    => out[n,c] = z[n]^T A_c z[n],  A_c = a_eff[:,c].reshape(4,4)

Device algorithm (per core, data-parallel over N, all-bf16 matmuls):
  Host stages x as Z[3g+j, m] = x[14 m + g, j] (G=14 groups x 3 comps) plus a
  row of ones (43 partition rows, points along the free dim, bf16).
  mm1:  V[127,F] = pv^T @ Z      9 forms per group + 1 shared unit row:
        x0,x1,x2, x0+x1,x0+x2,x1+x2, x0+1,x1+1,x2+1 (via the ones row), 1
  ACT/DVE: R = V^2 elementwise -> bf16 (no bias needed anywhere)
  mm2:  O = ab^T @ R   per group: out_c = sum_s W[c,s] R_s + K'_c
        (universal closed-form weights; constants folded via the unit row)
  Two consecutive tiles pack into one PSUM tile [106,512] at PE base
  partitions 0/64 (ab is zero-padded to 64 cols so rows 42:64 are written);
  one DVE copy drains each to bf16 SBUF; 9 per-super DMAs write DRAM.
  PE warm-up matmuls run during input DMA to raise the PE p-state early.
"""

import sys

if "/opt/trn_rl_repo" not in sys.path:
    sys.path.insert(0, "/opt/trn_rl_repo")

from contextlib import ExitStack

import ml_dtypes
import numpy as np

import concourse.bass as bass
import concourse.mybir as mybir
import concourse.tile as tile
from concourse import bacc
from concourse.bass_utils import run_bass_kernel_spmd

N_CORES = 8
N_POINTS = 1_000_000
N_PER_CORE = N_POINTS // N_CORES  # 125000
G = 14  # points (groups) per column
ZR = 3 * G + 1  # 43 partition rows of Z (incl. the ones row)
VR = 9 * G + 1  # 127 = form rows + shared unit row
T = 512  # matmul free-dim tile
NT = 18  # tiles per core
NM = NT // 2  # 9 macros (1024-col activation / output super-tiles)
FTOT = NT * T  # 9216 columns per core
NPAD = G * FTOT  # 129024 >= N_PER_CORE
ABW = 64  # ab stationary width; cols 42:64 are zero (pads rows 42:64)
OROW = 64 + 3 * G  # 106 rows in the packed output tile
# input DMA chunk column boundaries: tiny first chunk goes via the sync
# HW-DGE queue (lowest latency); the big rest via gpsimd SW-DGE (descriptors
# spread across all 16 physical DMA engines = high bandwidth)
CHB = [0, 512, 1024, 2048, 3584, 5120, 6656, 8192, 9216]
SYNC_CH = {0, 6}  # latency-critical first chunk + one late chunk ride DMA_0
NCH = len(CHB) - 1
WS = 924  # of each 1024-col macro, columns squared on Scalar (rest on DVE)
# output DMA chunks (in super-tiles): big early, small last to cut the tail
OCB = [0, 3, 6, 8, 9]
ODR = 3 * G  # 42 rows per packed output tensor (garbage rows dropped)

BF16 = ml_dtypes.bfloat16

_CACHE: dict = {}


def _build_nc():
    nc = bacc.Bacc("TRN2", target_bir_lowering=False)
    f32 = mybir.dt.float32
    bf16 = mybir.dt.bfloat16

    z_d = nc.dram_tensor("z", [ZR, FTOT], bf16, kind="ExternalInput")
    pv_d = nc.dram_tensor("pv", [ZR, VR], bf16, kind="ExternalInput")
    ab_d = nc.dram_tensor("ab", [VR, ABW], bf16, kind="ExternalInput")
    oa_d = nc.dram_tensor("oa", [ODR, NM * T], bf16, kind="ExternalOutput")
    ob_d = nc.dram_tensor("ob", [ODR, NM * T], bf16, kind="ExternalOutput")

    sq = mybir.ActivationFunctionType.Square
    add = mybir.AluOpType.add
    mult = mybir.AluOpType.mult

    with tile.TileContext(nc) as tc:
        with ExitStack() as ctx:
            cpool = ctx.enter_context(tc.tile_pool(name="consts", bufs=1))
            rpool = ctx.enter_context(tc.tile_pool(name="rt", bufs=2))
            tpool = ctx.enter_context(tc.tile_pool(name="tb", bufs=2))
            vpool = ctx.enter_context(
                tc.tile_pool(name="vps", bufs=1, space="PSUM")
            )
            opool = ctx.enter_context(
                tc.tile_pool(name="ops", bufs=1, space="PSUM")
            )
            pv = cpool.tile([ZR, VR], bf16)
            ab = cpool.tile([VR, ABW], bf16)
            oa_sb = cpool.tile([ODR, NM * T], bf16)
            ob_sb = cpool.tile([ODR, NM * T], bf16)
            zc = [
                cpool.tile([ZR, CHB[k + 1] - CHB[k]], bf16, name=f"zc{k}")
                for k in range(NCH)
            ]

            # DMA routing: three concurrent paths (each dma_start sustains
            # only ~25-30GB/s). gpsimd SW-DGE carries the early tiles +
            # consts; the sync and scalar HW-DGE queues each stream one late
            # block in parallel.
            nc.sync.dma_start(pv[:], pv_d[:, :])
            nc.sync.dma_start(zc[0][:], z_d[:, CHB[0] : CHB[1]])
            nc.sync.dma_start(ab[:], ab_d[:, :])
            for k in range(1, NCH):
                eng = nc.sync if k in SYNC_CH else nc.gpsimd
                eng.dma_start(zc[k][:], z_d[:, CHB[k] : CHB[k + 1]])

            # macro pairs: 4x mm1 (one pv weight load), squares, 4x mm2
            # (one ab load) -- halves PE LD_WEIGHTS thrash
            for mp in range(0, NM, 2):
                ms = [m for m in (mp, mp + 1) if m < NM]
                vt, rts, opst = {}, {}, {}
                for m in ms:
                    vt[m] = vpool.tile([VR, 2 * T], f32, name=f"vps{m % 3}")
                    for h in range(2):
                        c0 = (2 * m + h) * T
                        k = next(i for i in range(NCH) if CHB[i + 1] > c0)
                        o0 = c0 - CHB[k]
                        nc.tensor.matmul(
                            vt[m][:, h * T : (h + 1) * T],
                            pv[:],
                            zc[k][:, o0 : o0 + T],
                            start=True,
                            stop=True,
                        )
                for m in ms:
                    rt = rpool.tile([VR, 2 * T], bf16, name=f"rt{m % 2}")
                    rts[m] = rt
                    nc.scalar.activation(rt[:, :WS], vt[m][:, :WS], sq)
                    tb = tpool.tile([VR, 2 * T - WS], bf16, name=f"tb{m % 2}")
                    nc.vector.tensor_scalar(
                        tb[:], vt[m][:, WS:], 0.0, None, add
                    )
                    nc.vector.tensor_tensor(rt[:, WS:], tb[:], tb[:], mult)
                for m in ms:
                    ops = opool.tile([OROW, T], f32, name=f"ops{m % 2}")
                    opst[m] = ops
                    nc.tensor.matmul(
                        ops[0:ABW, :], ab[:], rts[m][:, 0:T],
                        start=True, stop=True,
                    )
                    nc.tensor.matmul(
                        ops[ABW:OROW, :], ab[:, : 3 * G],
                        rts[m][:, T : 2 * T], start=True, stop=True,
                    )
                for m in ms:
                    nc.vector.tensor_scalar(
                        oa_sb[:, m * T : (m + 1) * T],
                        opst[m][0 : 3 * G, :], 0.0, None, add,
                    )
                    nc.vector.tensor_scalar(
                        ob_sb[:, m * T : (m + 1) * T],
                        opst[m][ABW:OROW, :], 0.0, None, add,
                    )
                    oj = [
                        i for i in range(len(OCB) - 1) if OCB[i + 1] - 1 == m
                    ]
                    if oj:
                        j0, j1 = OCB[oj[0]] * T, OCB[oj[0] + 1] * T
                        nc.gpsimd.dma_start(oa_d[:, j0:j1], oa_sb[:, j0:j1])
                        nc.gpsimd.dma_start(ob_d[:, j0:j1], ob_sb[:, j0:j1])
    nc.compile()
    return nc


def _host_tensors(a: np.ndarray):
    """pv / ab from param a [96,3] (exact closed form, fp64)."""
    a_eff = a.astype(np.float64).reshape(6, 16, 3).sum(0)  # [16,3]
    A = a_eff.T.reshape(3, 4, 4)
    As = 0.5 * (A + A.transpose(0, 2, 1))
    Q = As[:, :3, :3]  # [3,3,3] quadratic part
    L = 2.0 * As[:, :3, 3]  # [3,3] linear coefs
    K = As[:, 3, 3]  # [3] constants

    pairs = [(0, 1), (0, 2), (1, 2)]
    W = np.zeros((3, 9))
    for c in range(3):
        for p, (j, k) in enumerate(pairs):
            W[c, 3 + p] = Q[c, j, k]
        for j in range(3):
            W[c, 6 + j] = 0.5 * L[c, j]
            W[c, j] = (
                Q[c, j, j]
                - sum(Q[c, j, k] for k in range(3) if k != j)
                - 0.5 * L[c, j]
            )
    Wones = K - 0.5 * L.sum(axis=1)  # [3]

    pv = np.zeros((ZR, VR), dtype=np.float32)
    ab = np.zeros((VR, ABW), dtype=np.float32)
    for g in range(G):
        for j in range(3):
            pv[3 * g + j, 9 * g + j] = 1.0  # x_j
            pv[3 * g + j, 9 * g + 6 + j] = 1.0  # x_j + 1 ...
            pv[ZR - 1, 9 * g + 6 + j] = 1.0  # ... via the ones row
        for p, (j, k) in enumerate(pairs):
            pv[3 * g + j, 9 * g + 3 + p] = 1.0  # x_j + x_k
            pv[3 * g + k, 9 * g + 3 + p] = 1.0
        for c in range(3):
            for ss in range(9):
                ab[9 * g + ss, 3 * g + c] = W[c, ss]
            ab[VR - 1, 3 * g + c] = Wones[c]
    pv[ZR - 1, VR - 1] = 1.0  # shared unit row
    return pv.astype(BF16), ab.astype(BF16)


def _stage_x(x: np.ndarray, ci: int) -> np.ndarray:
    xs = x[ci * N_PER_CORE : (ci + 1) * N_PER_CORE]
    xp = np.zeros((NPAD, 3), dtype=np.float32)
    xp[:N_PER_CORE] = xs
    z = np.empty((ZR, FTOT), dtype=np.float32)
    z[: ZR - 1] = xp.reshape(FTOT, G, 3).transpose(1, 2, 0).reshape(ZR - 1, FTOT)
    z[ZR - 1] = 1.0
    return z.astype(BF16)


def _decode_o(oa: np.ndarray, ob: np.ndarray) -> np.ndarray:
    """oa/ob [42, 4608] bf16 -> [N_PER_CORE, 3] fp32."""
    tmp = np.stack(
        [oa.astype(np.float32), ob.astype(np.float32)]
    )  # [b, 42, 4608]
    o5 = tmp.reshape(2, G, 3, NM, T)  # [b,g,c,s,w]
    full = o5.transpose(3, 0, 4, 1, 2).reshape(NPAD, 3)  # m = 1024s+512b+w
    return full[:N_PER_CORE]


def kernel(x: np.ndarray, a: np.ndarray) -> np.ndarray:
    x = np.ascontiguousarray(x, dtype=np.float32)
    a = np.ascontiguousarray(a, dtype=np.float32)
    if "nc" not in _CACHE:
        _CACHE["nc"] = _build_nc()
    nc = _CACHE["nc"]

    pv, ab = _host_tensors(a)
    in_maps = []
    for ci in range(N_CORES):
        in_maps.append({"z": _stage_x(x, ci), "pv": pv, "ab": ab})

    res = run_bass_kernel_spmd(nc, in_maps, list(range(N_CORES)))

    out = np.empty((N_POINTS, 3), dtype=np.float32)
    for ci in range(N_CORES):
        out[ci * N_PER_CORE : (ci + 1) * N_PER_CORE] = _decode_o(
            res.results[ci]["oa"], res.results[ci]["ob"]
        )
    return out
